# revision 57
# baseline (speedup 1.0000x reference)
"""Trainium2 Bass kernel for MultiHead GQA attention (B=1, S=2048, D=1024,
16 q-heads / 4 kv-heads, GQA group 4, RoPE, causal).  bf16 compute, f32 PSUM.

Sharding: tensor-parallel over heads. Core c (of 8) computes 2 query heads
{g, g+4} (c even) or {g+8, g+12} (c odd) with g = c//2, which both attend kv
head g (jnp.tile GQA semantics: q-head h uses kv head h % 4). Wq/Wk/Wv are
column-sharded, Wo row-sharded; each core produces a partial [D, S] bf16
output (transposed) and the host reduces the 8 partials and adds bo.

Per-core dataflow (activations kept in "transposed" [feature, seq] layout):
  - V and K projections run as one col-tiled matmul pass per (f, s):
    V -> PE col groups 0-1 (psum rows 0:64), K -> groups 2-3 (rows 64:128).
  - qh [128, S] = both heads stacked; khT2 [128, S] = kv head duplicated
    into both partition halves (rope on [64:128], then SBUF->SBUF DMA dup).
  - RoPE uses host-deinterleaved head_dim (evens then odds as partition
    blocks), so it is quadrant copies + 2 muls + 1 add on DVE.
  - scores: per (it, jt) the two heads run as row-tiled concurrent matmuls
    (tile_position (0,0) / (64,0)) into a 2-bank psum pair; one ACT exp
    (scale=1/8) emits bf16 probabilities for both heads.
  - causal mask applied structurally: sub-diagonal tiles skipped, diagonal
    128-chunks masked with a [128,128] tril tile on GpSimd.
  - PV accumulates [65, 512] per head (row 64 = softmax denominator via the
    ones column in vh_aug); DVE reciprocal_approx_fast of the denominator,
    DRAM-bounce broadcast, DVE multiply psum->attn bf16.
  - Output projection (row-shard of Wo) deferred one it-tile to hide the
    bounce round-trip; psum pairs drained by DVE/ACT to bf16, DMA'd out.
"""

import numpy as np
import ml_dtypes
from contextlib import ExitStack

import concourse.bass as bass
from concourse import bacc
import concourse.mybir as mybir
import concourse.tile as tile
from concourse.bass_utils import run_bass_kernel_spmd

f32 = mybir.dt.float32
bf16 = mybir.dt.bfloat16
MDT = bf16
NPBF = ml_dtypes.bfloat16

S = 2048
D = 1024
HEADS = 16
HD = 64
KVH = 4
N_CORES = 8

ST = 512          # i-tile (free dim of most matmuls)
NS = S // ST      # 4
FP = 128          # contraction chunk
NF = D // FP      # 8
JTS = 128         # j-chunk (key positions per score tile partition dim)
NJ = S // JTS     # 16
NE = D // 128     # 8 output-feature chunks

_CACHE = {}


def _build_program(debug=False):
    key = ("nc", debug)
    if key in _CACHE:
        return _CACHE[key]

    nc = bacc.Bacc("TRN2", target_bir_lowering=False, debug=False)

    def din(name, shape, dt=MDT):
        return nc.dram_tensor(name, shape, dt, kind="ExternalInput").ap()

    # inputs pre-chunked on host: [s-tile][partition][f * 512] contiguous
    qT = din("qT", [NS, 128, NF * ST])
    kT = din("kT", [NS, 128, NF * ST])
    vT = din("vT", [NS, 128, NF * ST])
    wq = din("wq", [128, NF * 128])
    wk = din("wk", [128, NF * 64])
    wv = din("wv", [128, NF * 64])
    wo = din("wo", [128, D])
    bq = din("bq", [128, 1], f32)
    bkv = din("bkv", [128, 1], f32)       # rows 0:64 = bv, 64:128 = bk
    cosk = din("cosk", [128, S])
    sink = din("sink", [128, S])
    tril = din("tril", [128, 128])
    ident_in = din("ident", [64, 64])
    outT = nc.dram_tensor("outT", [D, S], MDT, kind="ExternalOutput").ap()
    rcb = nc.dram_tensor("rcb", [2, S], MDT).ap()   # recip bounce (internal)

    Exp = mybir.ActivationFunctionType.Exp

    with tile.TileContext(nc) as tc, ExitStack() as ctx, \
            nc.allow_low_precision(reason="bf16 kernel by design"):
        const = ctx.enter_context(tc.tile_pool(name="const", bufs=1))
        big = ctx.enter_context(tc.tile_pool(name="big", bufs=1))
        stream = ctx.enter_context(tc.tile_pool(name="stream", bufs=1))
        ptile = ctx.enter_context(tc.tile_pool(name="ptile", bufs=1))
        small = ctx.enter_context(tc.tile_pool(name="small", bufs=1))
        outb = ctx.enter_context(tc.tile_pool(name="outb", bufs=1))
        psum = ctx.enter_context(tc.tile_pool(name="psum", bufs=1, space="PSUM"))

        def mm(out, lhsT, rhs, start, stop, tp=None, skip=False):
            nc.tensor.matmul(out, lhsT=lhsT, rhs=rhs, start=start, stop=stop,
                             tile_position=tp, skip_group_check=skip)

        # ---- constants: sync ring gets V/K weights (needed first); q input
        # chunks ride the gpsimd ring; tables/wo on the scalar ring ----
        ident = const.tile([64, 64], MDT)
        nc.sync.dma_start(out=ident, in_=ident_in)
        wv_sb = const.tile([128, NF, 64], MDT)
        nc.sync.dma_start(out=wv_sb, in_=wv.rearrange("p (f d) -> p f d", f=NF))
        wk_sb = const.tile([128, NF, 64], MDT)
        nc.sync.dma_start(out=wk_sb, in_=wk.rearrange("p (f d) -> p f d", f=NF))
        cos_sb = const.tile([128, S], MDT)
        sin_sb = const.tile([128, S], MDT)
        tril_sb = const.tile([128, 128], MDT)
        nc.scalar.dma_start(out=tril_sb, in_=tril)
        wo_sb = const.tile([128, D], MDT)
        nc.scalar.dma_start(out=wo_sb, in_=wo)
        wq_sb = const.tile([128, NF, 128], MDT)
        bq_sb = const.tile([128, 1], f32)
        bkv_sb = const.tile([128, 1], f32)

        qh = big.tile([128, S], MDT)
        khT2 = big.tile([128, S], MDT)
        vhT = big.tile([64, S], MDT)
        vh_aug = big.tile([128, NJ, 65], MDT)
        attn = big.tile([128, S], MDT)
        nc.vector.memset(vh_aug[:, :, 64], 1.0)
        ones16 = const.tile([1, 64], MDT)
        nc.vector.memset(ones16, 1.0)

        def oproj_units(it):
            isl = slice(it * ST, (it + 1) * ST)
            units = []
            for ep in range(NE // 2):
                def u(ep=ep):
                    pw = psum.tile([128, 2, ST], f32, tag="mm", bufs=2,
                                   name="pw")
                    for half in range(2):
                        e = 2 * ep + half
                        mm(pw[:, half, :], wo_sb[:, e * 128:(e + 1) * 128],
                           attn[:, isl], start=True, stop=True)
                    ob = outb.tile([128, 2, ST], MDT, tag="ob", bufs=3)
                    nc.scalar.copy(ob, pw)
                    nc.gpsimd.dma_start(
                        out=outT.rearrange("(g p) s -> p g s", p=128)
                        [:, 2 * ep:2 * ep + 2, isl],
                        in_=ob)
                units.append(u)
            return units

        def proj_dma(s):
            # ---- input streams for this s-tile (contiguous 1MB chunks) ----
            xv = stream.tile([128, NF, ST], MDT, tag="xv", bufs=3)
            xk = stream.tile([128, NF, ST], MDT, tag="xk", bufs=3)
            xq = stream.tile([128, NF, ST], MDT, tag="xq", bufs=3)
            if s == 0:
                # split first chunks so matmuls can start on the first half
                for h in range(2):
                    fs = slice(4 * h, 4 * h + 4)
                    nc.sync.dma_start(
                        out=xv[:, fs, :],
                        in_=vT[s].rearrange("p (f c) -> p f c", f=NF)[:, fs, :])
                    nc.sync.dma_start(
                        out=xk[:, fs, :],
                        in_=kT[s].rearrange("p (f c) -> p f c", f=NF)[:, fs, :])
                nc.gpsimd.dma_start(
                    out=xq, in_=qT[s].rearrange("p (f c) -> p f c", f=NF))
                # after the first input chunks: rope tables + Q weights
                nc.sync.dma_start(out=cos_sb, in_=cosk)
                nc.sync.dma_start(out=sin_sb, in_=sink)
                nc.sync.dma_start(
                    out=wq_sb, in_=wq.rearrange("p (f d) -> p f d", f=NF))
                nc.sync.dma_start(out=bq_sb, in_=bq)
                nc.sync.dma_start(out=bkv_sb, in_=bkv)
            else:
                nc.sync.dma_start(
                    out=xv, in_=vT[s].rearrange("p (f c) -> p f c", f=NF))
                nc.sync.dma_start(
                    out=xk, in_=kT[s].rearrange("p (f c) -> p f c", f=NF))
                nc.gpsimd.dma_start(
                    out=xq, in_=qT[s].rearrange("p (f c) -> p f c", f=NF))
            return xv, xk, xq

        def proj_units(s, xv, xk, xq, st):
            # matmul work units; st carries the psum tiles to proj_finish
            units = []
            for f in range(NF):
                def ukv(f=f):
                    if "kv" not in st:
                        st["kv"] = psum.tile([128, ST], f32, tag="acc",
                                             bufs=4, name="pskv")
                    mm(st["kv"][0:64, :], wv_sb[:, f, :], xv[:, f, :],
                       start=(f == 0), stop=(f == NF - 1), tp=(0, 0))
                    mm(st["kv"][64:128, :], wk_sb[:, f, :], xk[:, f, :],
                       start=(f == 0), stop=(f == NF - 1), tp=(0, 64),
                       skip=True)
                units.append(ukv)
            for f in range(NF):
                def uq(f=f):
                    if "q" not in st:
                        st["q"] = psum.tile([128, ST], f32, tag="acc",
                                            bufs=4, name="psq")
                    mm(st["q"], wq_sb[:, f, :], xq[:, f, :],
                       start=(f == 0), stop=(f == NF - 1))
                units.append(uq)
            return units

        def proj_finish(s, st):
            ssl = slice(s * ST, (s + 1) * ST)
            ps_kv = st["kv"]
            ps_q = st["q"]
            # q first: the next attention tile's early jt-slots need only
            # roped q (roped k / vh_aug are used by the late diagonal slots);
            # q drain on DVE so it never queues behind ACT's ob-copy backlog
            Iden = mybir.ActivationFunctionType.Identity
            nc.vector.tensor_scalar_add(qh[:, ssl], ps_q, bq_sb)
            qsw = ptile.tile([128, ST], MDT, tag="qsw", bufs=2)
            for (dstp, srcp) in ((0, 32), (32, 0), (64, 96), (96, 64)):
                nc.vector.tensor_copy(qsw[dstp:dstp + 32, :],
                                      qh[srcp:srcp + 32, ssl])
            nc.vector.tensor_mul(qsw, qsw, sin_sb[:, ssl])
            nc.vector.tensor_mul(qh[:, ssl], qh[:, ssl], cos_sb[:, ssl])
            nc.vector.tensor_add(qh[:, ssl], qh[:, ssl], qsw)

            nc.scalar.activation(out=vhT[:, ssl], in_=ps_kv[0:64, :],
                                 func=Iden, bias=bkv_sb[0:64, :])
            nc.scalar.activation(out=khT2[64:128, ssl], in_=ps_kv[64:128, :],
                                 func=Iden, bias=bkv_sb[64:128, :])

            # ---- transpose V to [seq, dim] in vh_aug (ones col = denom) ----
            for m in range(4):
                jt = 4 * s + m
                tp_ps = psum.tile([128, 2, ST], MDT, tag="mm", bufs=2, name="tp")
                nc.tensor.transpose(tp_ps[:, 0, 0:64],
                                    vhT[:, jt * JTS:(jt + 1) * JTS], ident)
                nc.vector.tensor_copy(vh_aug[:, jt, 0:64], tp_ps[:, 0, 0:64])

            # ---- RoPE (pairs are 32-partition blocks; swap + 2 mul + add) ----
            ksw = ptile.tile([128, ST], MDT, tag="ksw", bufs=2)
            nc.vector.tensor_copy(ksw[64:96, :], khT2[96:128, ssl])
            nc.vector.tensor_copy(ksw[96:128, :], khT2[64:96, ssl])
            nc.vector.tensor_mul(ksw[64:128, :], ksw[64:128, :],
                                 sin_sb[64:128, ssl])
            nc.vector.tensor_mul(khT2[64:128, ssl], khT2[64:128, ssl],
                                 cos_sb[64:128, ssl])
            nc.vector.tensor_add(khT2[64:128, ssl], khT2[64:128, ssl],
                                 ksw[64:128, :])
            # duplicate roped kv head into partitions 0:64 for head-0 scores
            nc.gpsimd.dma_start(out=khT2[0:64, ssl], in_=khT2[64:128, ssl])

        def attn_block(it, feed=()):
            # ---- attention for it (both heads, row-tiled scores); `feed`
            # units (projection/output matmuls) are woven between jt slots so
            # the PE works in the exp shadows and ACT never starves ----
            feed = list(feed)
            po0 = psum.tile([65, ST], f32, tag="acc", bufs=4, name="po0")
            po1 = psum.tile([65, ST], f32, tag="acc", bufs=4, name="po1")
            jmax = 4 * it + 3
            for jt in range(jmax + 1):
                if feed and jt >= 1:
                    n = -(-len(feed) // (jmax + 1 - jt))
                    for u in feed[:n]:
                        u()
                    feed = feed[n:]
                lo = (jt - 4 * it) * JTS if jt >= 4 * it else 0
                jsl = slice(jt * JTS, (jt + 1) * JTS)
                isl = slice(it * ST + lo, (it + 1) * ST)
                pair = psum.tile([128, 2, ST], f32, tag="mm", bufs=2,
                                 name="pair")
                mm(pair[:, 0, lo:], khT2[0:64, jsl], qh[0:64, isl],
                   start=True, stop=True, tp=(0, 0))
                mm(pair[:, 1, lo:], khT2[64:128, jsl], qh[64:128, isl],
                   start=True, stop=True, tp=(64, 0))
                pt = ptile.tile([128, 2, ST], MDT, tag="pt", bufs=3)
                nc.scalar.activation(out=pt[:, :, lo:], in_=pair[:, :, lo:],
                                     func=Exp, scale=0.125)
                if jt >= 4 * it:
                    for half in range(2):
                        nc.vector.tensor_mul(pt[:, half, lo:lo + JTS],
                                             pt[:, half, lo:lo + JTS], tril_sb)
                if debug and it == 1 and jt == 2:
                    dpt = nc.dram_tensor("d_pt", [128, 2 * ST], MDT,
                                         kind="ExternalOutput").ap()
                    nc.sync.dma_start(
                        out=dpt.rearrange("p (a b) -> p a b", a=2), in_=pt)
                mm(po0[:, lo:], vh_aug[:, jt, :], pt[:, 0, lo:],
                   start=(jt == 0), stop=(jt == jmax))
                mm(po1[:, lo:], vh_aug[:, jt, :], pt[:, 1, lo:],
                   start=(jt == 0), stop=(jt == jmax))

            # ---- softmax denominators -> bf16 reciprocals -> broadcast ----
            isl = slice(it * ST, (it + 1) * ST)
            sums = small.tile([1, 2, ST], f32, tag="sums", bufs=2)
            rc = small.tile([1, 2, ST], f32, tag="rc", bufs=2)
            rcb16 = small.tile([1, 2, ST], MDT, tag="rcb16", bufs=2)
            nc.scalar.copy(sums[:, 0, :], po0[64:65, :])
            nc.scalar.copy(sums[:, 1, :], po1[64:65, :])
            nc.vector.reciprocal_approx_fast(rc, sums)
            nc.vector.tensor_copy(rcb16, rc)
            bct = ptile.tile([128, ST], MDT, tag="bct", bufs=2)
            if it == NS - 1:
                # last tile: PE broadcast (K=1 matmul) avoids the DRAM-bounce
                # round trip sitting on the critical path of the tail
                bp = psum.tile([128, 2, ST], f32, tag="mm", bufs=2, name="bp")
                mm(bp[0:64, 0, :], ones16, rcb16[:, 0, :],
                   start=True, stop=True, tp=(0, 0))
                mm(bp[64:128, 0, :], ones16, rcb16[:, 1, :],
                   start=True, stop=True, tp=(0, 64), skip=True)
                nc.vector.tensor_copy(bct, bp[:, 0, :])
            else:
                nc.scalar.dma_start(out=rcb[0:1, isl], in_=rcb16[:, 0, :])
                nc.scalar.dma_start(out=rcb[1:2, isl], in_=rcb16[:, 1, :])
                for h in range(2):
                    rsrc = rcb[h:h + 1, isl]
                    rsrc = bass.AP(tensor=rsrc.tensor, offset=rsrc.offset,
                                   ap=[[0, 64]] + list(rsrc.ap)[1:])
                    nc.gpsimd.dma_start(out=bct[h * 64:(h + 1) * 64, :],
                                        in_=rsrc)
            if debug and it == 1:
                dbc = nc.dram_tensor("d_bct", [128, ST], MDT,
                                     kind="ExternalOutput").ap()
                nc.sync.dma_start(out=dbc, in_=bct)
            nc.vector.tensor_mul(attn[0:64, isl], po0[0:64, :], bct[0:64, :])
            nc.vector.tensor_mul(attn[64:128, isl], po1[0:64, :],
                                 bct[64:128, :])

        # warm up the PE during the initial input-DMA wait (identity
        # transposes keep HAM at full clock until real matmuls arrive)
        for _ in range(64):
            wt = psum.tile([128, 2, ST], MDT, tag="mm", bufs=2, name="warm")
            nc.tensor.transpose(wt[0:64, 0, 0:64], ident, ident)

        # software pipeline: attention lags projections by one s-tile,
        # output projection lags attention by one more; projection and
        # output-projection matmuls are fed between attention slots
        for s in range(NS):
            xv, xk, xq = proj_dma(s)
            st = {}
            for u in proj_units(s, xv, xk, xq, st):
                u()
            proj_finish(s, st)
            if s >= 1:
                attn_block(s - 1)
            if s >= 2:
                for u in oproj_units(s - 2):
                    u()
        attn_block(NS - 1, feed=oproj_units(NS - 2))
        for u in oproj_units(NS - 1):
            u()

        if debug:
            dqh = nc.dram_tensor("d_qh", [128, S], MDT,
                                 kind="ExternalOutput").ap()
            dkh = nc.dram_tensor("d_khT2", [128, S], MDT,
                                 kind="ExternalOutput").ap()
            dvh = nc.dram_tensor("d_vhT", [64, S], MDT,
                                 kind="ExternalOutput").ap()
            dva = nc.dram_tensor("d_vh_aug", [128, NJ * 65], MDT,
                                 kind="ExternalOutput").ap()
            dat = nc.dram_tensor("d_attn", [128, S], MDT,
                                 kind="ExternalOutput").ap()
            nc.sync.dma_start(out=dqh, in_=qh)
            nc.sync.dma_start(out=dkh, in_=khT2)
            nc.sync.dma_start(out=dvh, in_=vhT)
            nc.sync.dma_start(
                out=dva.rearrange("p (j e) -> p j e", j=NJ), in_=vh_aug)
            nc.sync.dma_start(out=dat, in_=attn)

    nc.compile()
    _CACHE[key] = nc
    return nc


def _host_tables():
    if "tables" in _CACHE:
        return _CACHE["tables"]
    # faithful to reference: exp = -2*arange(0,64,2)/64
    expv = -2.0 * np.arange(0, HD, 2, dtype=np.float32) / HD
    thetas = np.power(np.float32(10000.0), expv).astype(np.float32)    # [32]
    m = np.arange(S, dtype=np.float32)
    freq = np.outer(m, thetas).astype(np.float32)                      # [S, 32]
    cos = np.cos(freq).astype(np.float32).T                            # [32, S]
    sin = np.sin(freq).astype(np.float32).T
    cos128 = np.concatenate([cos, cos, cos, cos], 0)                   # [128, S]
    sin128 = np.concatenate([-sin, sin, -sin, sin], 0)
    perm = np.concatenate([np.arange(0, HD, 2), np.arange(1, HD, 2)])  # deint
    trilm = (np.arange(128)[:, None] <= np.arange(128)[None, :])
    _CACHE["tables"] = (
        np.ascontiguousarray(cos128.astype(NPBF)),
        np.ascontiguousarray(sin128.astype(NPBF)),
        perm,
        np.ascontiguousarray(trilm.astype(NPBF)),
    )
    return _CACHE["tables"]


def _warr(w):
    # [1024, nd] -> [128, NF*nd] with chunk-of-128-rows as middle dim
    nd = w.shape[1]
    return np.ascontiguousarray(
        w.reshape(NF, FP, nd).transpose(1, 0, 2).reshape(FP, NF * nd)
        .astype(NPBF))


def kernel(**inputs):
    q = np.asarray(inputs["q"], np.float32)[0]       # [S, D]
    k = np.asarray(inputs["k"], np.float32)[0]
    v = np.asarray(inputs["v"], np.float32)[0]
    Wq = np.asarray(inputs["Wq"], np.float32)
    Wk = np.asarray(inputs["Wk"], np.float32)
    Wv = np.asarray(inputs["Wv"], np.float32)
    Wo = np.asarray(inputs["Wo"], np.float32)
    bq = np.asarray(inputs["bq"], np.float32)
    bk = np.asarray(inputs["bk"], np.float32)
    bv = np.asarray(inputs["bv"], np.float32)
    bo = np.asarray(inputs["bo"], np.float32)

    cos128, sin128, perm, trilm = _host_tables()

    # head_dim deinterleave permutation applied to q/k projection columns
    permQ = np.concatenate([h * HD + perm for h in range(HEADS)])
    permK = np.concatenate([g * HD + perm for g in range(KVH)])
    Wqp = Wq[:, permQ]
    bqp = bq[permQ]
    Wkp = Wk[:, permK]
    bkp = bk[permK]

    def chunk(x):
        # [S, D] -> [NS, 128, NF*512]: xc[s, p, f*512+c] = x[s*512+c, f*128+p]
        xc = x.T.reshape(NF, FP, NS, ST).transpose(2, 1, 0, 3)
        return np.ascontiguousarray(
            xc.reshape(NS, FP, NF * ST).astype(NPBF))

    qT = chunk(q)
    kT = chunk(k)
    vT = chunk(v)
    ident64 = np.eye(64, dtype=np.float32).astype(NPBF)

    in_maps = []
    for c in range(N_CORES):
        g = c // 2
        if c % 2 == 0:
            h0, h1 = g, g + 4
        else:
            h0, h1 = g + 8, g + 12
        wq_c = np.concatenate([Wqp[:, h0 * HD:(h0 + 1) * HD],
                               Wqp[:, h1 * HD:(h1 + 1) * HD]], axis=1)
        bq_c = np.ascontiguousarray(
            np.concatenate([bqp[h0 * HD:(h0 + 1) * HD],
                            bqp[h1 * HD:(h1 + 1) * HD]]).reshape(128, 1))
        bkv_c = np.ascontiguousarray(
            np.concatenate([bv[g * HD:(g + 1) * HD],
                            bkp[g * HD:(g + 1) * HD]]).reshape(128, 1))
        wo_c = np.ascontiguousarray(
            np.concatenate([Wo[h0 * HD:(h0 + 1) * HD, :],
                            Wo[h1 * HD:(h1 + 1) * HD, :]], axis=0)
            .astype(NPBF))

        in_maps.append({
            "qT": qT, "kT": kT, "vT": vT,
            "wq": _warr(wq_c),
            "wk": _warr(Wkp[:, g * HD:(g + 1) * HD]),
            "wv": _warr(Wv[:, g * HD:(g + 1) * HD]),
            "wo": wo_c,
            "bq": bq_c,
            "bkv": bkv_c,
            "cosk": cos128, "sink": sin128, "tril": trilm,
            "ident": ident64,
        })

    nc = _build_program()
    res = run_bass_kernel_spmd(nc, in_maps, list(range(N_CORES)))
    acc = np.zeros((D, S), np.float32)
    for r in res.results:
        acc += np.asarray(r["outT"], np.float32)
    out = acc.T + bo[None, :]
    return out[None].astype(np.float32)
